# revision 1
# baseline (speedup 1.0000x reference)
"""Trainium2 Bass kernel: BinarizedLinear  out = x @ (u < weight).T

Shapes (hardcoded): x [16384, 4096] f32, weight/u [512, 4096] f32,
out [16384, 512] f32.

Sharding: data-parallel over 8 NeuronCores — x sharded along batch
(2048 rows/core), weight/u replicated, no collectives; host concatenates
the per-core outputs.

Per-core kernel (Tile framework):
  Phase A: load weight/u (fp32), binarize on DVE (u < weight -> bf16
           {0,1}), xbar-DMA-transpose to wbt[i_local, k, o] so the
           contraction dim (INUM) is on partitions. wbt stays resident
           in SBUF (4 MB).
  Phase B: per 128-row batch tile: SWDGE cast-load x fp32->bf16,
           xbar-DMA-transpose to xt[i_local, k, b_local], then 32
           accumulating PE matmuls (bf16 in, fp32 PSUM) per output
           tile [128 b, 512 o], DVE copy PSUM->SBUF, store.

bf16 is used for the matmul operands: fp32 matmul runs at 4 cycles/row
on TRN2 while bf16 runs at 1; the fp32 PSUM accumulation keeps the
error vs the fp32 reference at ~1e-5 relative.
"""

import numpy as np

from concourse import bass, bacc, mybir, tile
from concourse.bass_utils import run_bass_kernel_spmd

B, INUM, ONUM = 16384, 4096, 512
NCORES = 8
BLOC = B // NCORES  # 2048 batch rows per core
P = 128             # partitions
NK = INUM // P      # 32 contraction tiles
NOT = ONUM // P     # 4 weight-row tiles

F32 = mybir.dt.float32
BF16 = mybir.dt.bfloat16

_CACHE = {}


def build(bloc=BLOC, gb=2, xn_bufs=3, xt_bufs=4, ob_bufs=4, ps_bufs=8,
          store_gb=1, loop=None, pe_groups=()):
    """gb: batch tiles (of 128 rows) grouped per x load/transpose DMA.

    loop: if set, wrap phase B in a For_i repeating it `loop` times
    (timing variant: same data each iteration, outputs overwritten).

    pe_groups: group indices whose x-transpose runs on the tensor engine
    (identity matmul -> bf16 PSUM -> DVE copy) instead of the DMA xbar,
    rebalancing SDMA-engine work onto PE idle time."""
    nbt = bloc // P
    ngrp = nbt // gb
    nc = bacc.Bacc("TRN2", target_bir_lowering=False, debug=False,
                   num_devices=NCORES)
    x_d = nc.dram_tensor("x", [bloc, INUM], F32, kind="ExternalInput")
    w_d = nc.dram_tensor("weight", [ONUM, INUM], F32, kind="ExternalInput")
    u_d = nc.dram_tensor("u", [ONUM, INUM], F32, kind="ExternalInput")
    o_d = nc.dram_tensor("out", [bloc, ONUM], F32, kind="ExternalOutput")

    # DRAM views, partition-major: x_v[g][p, j, i] = x[(g*gb + j)*P + p, i]
    x_v = x_d[:, :].rearrange("(g j p) i -> g p j i", g=ngrp, j=gb, p=P)
    o_v = o_d[:, :].rearrange("(g j p) o -> g p j o", g=nbt // store_gb,
                              j=store_gb, p=P)

    if pe_groups:
        ps_bufs = min(ps_bufs, 6)   # leave 2 PSUM banks for PE transposes

    with tile.TileContext(nc) as tc:
        with (
            tc.tile_pool(name="wbt", bufs=1) as wbt_pool,
            tc.tile_pool(name="ps", bufs=ps_bufs, space="PSUM") as ps_pool,
        ):
            from contextlib import ExitStack
            _aux = ExitStack()
            ident = None
            if pe_groups:
                from concourse import masks
                ident_pool = _aux.enter_context(
                    tc.tile_pool(name="ident", bufs=1))
                ident = ident_pool.tile([P, P], BF16)
                masks.make_identity(nc, ident[:])
            # ---- Phase A: binarized, transposed weights (resident) ----
            # wbt[i_local, k, o] = (u < weight)[o, k*128 + i_local]
            # One 2MB DMA per o-tile per tensor (chunking these into 32
            # small DMAs put ~80us of serialized SWDGE latency on the
            # critical path gating the first matmul); w rides SWDGE while
            # u rides the scalar HWDGE so descriptor generation overlaps.
            # wu/wb pools are phase-A-scoped; SBUF is reclaimed for B.
            wbt = wbt_pool.tile([P, NK, ONUM], BF16)
            with (
                tc.tile_pool(name="wu", bufs=2) as wu_pool,
                tc.tile_pool(name="wb", bufs=2) as wb_pool,
            ):
                for ot in range(NOT):
                    wb_t = wb_pool.tile([P, INUM], BF16, tag="wb")
                    w_t = wu_pool.tile([P, INUM], F32, tag="w")
                    u_t = wu_pool.tile([P, INUM], F32, tag="u")
                    nc.gpsimd.dma_start(out=w_t[:],
                                        in_=w_d[ot * P:(ot + 1) * P, :])
                    nc.scalar.dma_start(out=u_t[:],
                                        in_=u_d[ot * P:(ot + 1) * P, :])
                    nc.vector.tensor_tensor(wb_t[:], u_t[:], w_t[:],
                                            op=mybir.AluOpType.is_lt)
                    nc.sync.dma_start(out=wbt[:, :, ot * P:(ot + 1) * P],
                                      in_=wb_t[:], transpose=True)

            # ---- Phase B: stream batch tiles, gb tiles per DMA group ----
            with (
                tc.tile_pool(name="xn", bufs=xn_bufs) as xn_pool,
                tc.tile_pool(name="xt", bufs=xt_bufs) as xt_pool,
                tc.tile_pool(name="ob", bufs=ob_bufs) as ob_pool,
            ):
                pst_pool = None
                if pe_groups:
                    pst_pool = _aux.enter_context(
                        tc.tile_pool(name="pst", bufs=2, space="PSUM"))
                QT = 4   # k-tiles per PE-transpose PSUM staging quad

                def run_groups(_iv=None, prefetch=2):
                    # software-pipelined emission: group g+prefetch's
                    # load+transpose are emitted before group g's matmuls,
                    # so the prefetch DMAs outrank compute in the
                    # scheduler's priority order
                    xts = {}

                    def emit_fetch(g):
                        # xn[p, j, i] = x[(g*gb + j)*P + p, i]
                        xn = xn_pool.tile([P, gb, INUM], BF16, tag="xn")
                        nc.gpsimd.dma_start(out=xn[:], in_=x_v[g])
                        # xt[p, j*NK + k, f] = xn_2d[f, j*INUM + k*P + p]
                        #                    = x[(g*gb + j)*P + f, k*P + p]
                        xt = xt_pool.tile([P, gb * NK, P], BF16, tag="xt")
                        emit_transpose(g, xn, xt)
                        xts[g] = xt

                    ob = None
                    for g in range(min(prefetch + 1, ngrp)):
                        emit_fetch(g)
                    for g in range(ngrp):
                        xt = xts.pop(g)
                        for j in range(gb):
                            bt = g * gb + j
                            jj = bt % store_gb
                            if jj == 0:
                                ob = ob_pool.tile([P, store_gb, ONUM], F32,
                                                  tag="ob")
                            ps = ps_pool.tile([P, ONUM], F32, tag="ps")
                            for k in range(NK):
                                nc.tensor.matmul(ps[:], xt[:, j * NK + k, :],
                                                 wbt[:, k, :],
                                                 start=(k == 0),
                                                 stop=(k == NK - 1))
                            nc.vector.tensor_copy(ob[:, jj, :], ps[:])
                            if jj == store_gb - 1:
                                nc.scalar.dma_start(out=o_v[bt // store_gb],
                                                    in_=ob[:])
                        if g + prefetch + 1 < ngrp:
                            emit_fetch(g + prefetch + 1)

                def emit_transpose(g, xn, xt):
                    if g in pe_groups:
                        # tensor-engine transpose: each [128b,128i]
                        # sub-tile via identity matmul into bf16 PSUM,
                        # then DVE copies a quad back to SBUF
                        for j in range(gb):
                            for q in range(NK // QT):
                                pst = pst_pool.tile([P, QT, P], BF16,
                                                    tag="pst")
                                for h in range(QT):
                                    k = q * QT + h
                                    nc.tensor.transpose(
                                        pst[:, h, :],
                                        xn[:, j, k * P:(k + 1) * P],
                                        ident[:])
                                kk = j * NK + q * QT
                                nc.vector.tensor_copy(
                                    xt[:, kk:kk + QT, :], pst[:])
                    else:
                        nc.sync.dma_start(out=xt[:], in_=xn[:],
                                          transpose=True)

                if loop is None:
                    run_groups()
                else:
                    with tc.For_i(0, loop, 1):
                        run_groups()
            _aux.close()   # LIFO: release pst/ident after xn/xt/ob

    nc.compile()
    return nc


def _make_exec(nc):
    """Build a jitted shard_map executable over the 8 cores (mirrors
    bass2jax.run_bass_via_pjrt's multi-core path, without donation so the
    same device buffers can be re-executed for timing)."""
    import jax
    from jax.sharding import Mesh, PartitionSpec
    from jax.experimental.shard_map import shard_map
    from concourse import bass2jax

    bass2jax.install_neuronx_cc_hook()
    partition_name = (nc.partition_id_tensor.name
                      if nc.partition_id_tensor else None)
    in_names, out_names, out_avals = [], [], []
    for alloc in nc.m.functions[0].allocations:
        if not isinstance(alloc, mybir.MemoryLocationSet):
            continue
        name = alloc.memorylocations[0].name
        if alloc.kind == "ExternalInput":
            if name != partition_name:
                in_names.append(name)
        elif alloc.kind == "ExternalOutput":
            out_names.append(name)
            out_avals.append(jax.core.ShapedArray(
                tuple(alloc.tensor_shape), mybir.dt.np(alloc.dtype)))
    n_params = len(in_names)
    all_names = in_names + out_names
    if partition_name is not None:
        all_names = all_names + [partition_name]

    def _body(*args):
        operands = list(args)
        if partition_name is not None:
            operands.append(bass2jax.partition_id_tensor())
        return tuple(bass2jax._bass_exec_p.bind(
            *operands,
            out_avals=tuple(out_avals),
            in_names=tuple(all_names),
            out_names=tuple(out_names),
            lowering_input_output_aliases=(),
            sim_require_finite=True,
            sim_require_nnan=True,
            nc=nc,
        ))

    devices = jax.devices()[:NCORES]
    mesh = Mesh(np.asarray(devices), ("core",))

    def make_fn(reps):
        def _rep_body(*args):
            outs = None
            for _ in range(reps):
                outs = _body(*args)   # effectful primitive: not CSE'd
            return outs
        return jax.jit(
            shard_map(_rep_body, mesh=mesh,
                      in_specs=(PartitionSpec("core"),) * (n_params + len(out_names)),
                      out_specs=(PartitionSpec("core"),) * len(out_names),
                      check_rep=False),
            keep_unused=True,
        )

    return make_fn, mesh, in_names[:n_params], out_names, out_avals


def bench(x, weight, u, r_lo=32, r_hi=1024, iters=6, **build_kw):
    """Measure real device time for one kernel execution.

    The axon RPC jitter (tens of ms) swamps a single ~250us execution, and
    multiple identical bass_exec calls in one program get CSE'd. So we
    build two NEFF variants whose phase B repeats in an on-device For_i
    loop (r_lo and r_hi iterations) and difference the wall-clock minima:
    (t_hi - t_lo)/(r_hi - r_lo) is one full phase-B pass of device time.
    Phase A (binarize+transpose weights, ~25us, runs once) is added from
    its cost-model share."""
    import time
    import jax
    from jax.sharding import NamedSharding, PartitionSpec

    concat = {
        "x": np.ascontiguousarray(x, dtype=np.float32),
        "weight": np.concatenate([weight] * NCORES, axis=0),
        "u": np.concatenate([u] * NCORES, axis=0),
    }

    def run_variant(r):
        nc = build(loop=r, **build_kw)
        make_fn, mesh, in_names, out_names, out_avals = _make_exec(nc)
        sh = NamedSharding(mesh, PartitionSpec("core"))
        args = [jax.device_put(concat[n], sh) for n in in_names]
        zeros = [
            jax.device_put(
                np.zeros((NCORES * a.shape[0], *a.shape[1:]), a.dtype), sh)
            for a in out_avals
        ]
        fn = make_fn(1)
        jax.block_until_ready(fn(*args, *zeros))    # compile + warm
        best = float("inf")
        for _ in range(iters):
            t0 = time.perf_counter()
            jax.block_until_ready(fn(*args, *zeros))
            best = min(best, time.perf_counter() - t0)
        return best

    t_lo = run_variant(r_lo)
    t_hi = run_variant(r_hi)
    pass_ns = (t_hi - t_lo) / (r_hi - r_lo) * 1e9
    phase_a_ns = 25_000.0   # one-time weight binarize+transpose (cost model)
    print(f"bench: loop{r_lo}={t_lo*1e3:.1f}ms loop{r_hi}={t_hi*1e3:.1f}ms "
          f"-> phase-B pass {pass_ns/1e3:.1f}us + phase-A ~{phase_a_ns/1e3:.0f}us")
    return pass_ns + phase_a_ns


def kernel(x, weight, u):
    x = np.ascontiguousarray(np.asarray(x), dtype=np.float32)
    weight = np.ascontiguousarray(np.asarray(weight), dtype=np.float32)
    u = np.ascontiguousarray(np.asarray(u), dtype=np.float32)
    assert x.shape == (B, INUM) and weight.shape == (ONUM, INUM)

    nc = _CACHE.get("nc")
    if nc is None:
        nc = _CACHE["nc"] = build()

    in_maps = [
        {"x": x[c * BLOC:(c + 1) * BLOC], "weight": weight, "u": u}
        for c in range(NCORES)
    ]
    res = run_bass_kernel_spmd(nc, in_maps, list(range(NCORES)))
    return np.concatenate([res.results[c]["out"] for c in range(NCORES)],
                          axis=0)



# revision 7
# speedup vs baseline: 1.3030x; 1.3030x over previous
"""Trainium2 Bass kernel: BinarizedLinear  out = x @ (u < weight).T

Shapes (hardcoded): x [16384, 4096] f32, weight/u [512, 4096] f32,
out [16384, 512] f32.

Sharding: data-parallel over 8 NeuronCores — x sharded along batch
(2048 rows/core), weight/u replicated, no collectives; host concatenates
the per-core outputs.

Per-core kernel (Tile framework):
  Phase A: load weight/u (fp32), binarize on DVE (u < weight -> bf16
           {0,1}), xbar-DMA-transpose to wbt[i_local, k, o] so the
           contraction dim (INUM) is on partitions, then (fp8 path)
           cast bf16 -> fp8e4. wbt stays resident in SBUF.
  Phase B: per 128-row batch tile: SWDGE cast-load x fp32->bf16,
           xbar-DMA-transpose to xt[i_local, k, b_local], (fp8 path)
           cast bf16 -> fp8e4 split across DVE+ACT, then accumulating
           PE matmuls per output tile [128 b, 512 o] (fp8e4 DoubleRow:
           two k-tiles per instruction at 0.5 cycles/row, fp32 PSUM),
           DVE copy PSUM->SBUF, store.

fp8e4 ({0,1} weights are exact; x in [0,1) quantizes at ~3% RMS per
element, averaging out to ~1e-3 relative on the 4096-term dot product,
well inside the 2e-2 gate) doubles PE matmul throughput vs bf16.
"""

import numpy as np

from concourse import bass, bacc, mybir, tile
from concourse.bass_utils import run_bass_kernel_spmd

B, INUM, ONUM = 16384, 4096, 512
NCORES = 8
BLOC = B // NCORES  # 2048 batch rows per core
P = 128             # partitions
NK = INUM // P      # 32 contraction tiles
NOT = ONUM // P     # 4 weight-row tiles

F32 = mybir.dt.float32
BF16 = mybir.dt.bfloat16
F8 = mybir.dt.float8e4

_CACHE = {}


def build(bloc=BLOC, gb=2, xn_bufs=3, xt_bufs=4, ob_bufs=4, ps_bufs=8,
          store_gb=1, loop=None, pe_groups=(), fp8=True, x8_bufs=3):
    """gb: batch tiles (of 128 rows) grouped per x load/transpose DMA.

    loop: if set, wrap phase B in a For_i repeating it `loop` times
    (timing variant: same data each iteration, outputs overwritten).

    pe_groups: group indices whose x-transpose runs on the tensor engine
    (identity matmul -> bf16 PSUM -> DVE copy) instead of the DMA xbar,
    rebalancing SDMA-engine work onto PE idle time.

    fp8: cast both matmul operands bf16 -> fp8e4 (DVE/ACT split for x)
    and run the PE in DoubleRow mode (2 k-tiles per instruction)."""
    nbt = bloc // P
    ngrp = nbt // gb
    NKP = INUM // 256    # fp8 paired k'-tiles (256 contraction per matmul)
    nc = bacc.Bacc("TRN2", target_bir_lowering=False, debug=False,
                   num_devices=NCORES)
    x_d = nc.dram_tensor("x", [bloc, INUM], F32, kind="ExternalInput")
    w_d = nc.dram_tensor("weight", [ONUM, INUM], F32, kind="ExternalInput")
    u_d = nc.dram_tensor("u", [ONUM, INUM], F32, kind="ExternalInput")
    o_d = nc.dram_tensor("out", [bloc, ONUM], F32, kind="ExternalOutput")

    # DRAM views, partition-major: x_v[g][p, j, i] = x[(g*gb + j)*P + p, i]
    x_v = x_d[:, :].rearrange("(g j p) i -> g p j i", g=ngrp, j=gb, p=P)
    o_v = o_d[:, :].rearrange("(g j p) o -> g p j o", g=nbt // store_gb,
                              j=store_gb, p=P)

    if pe_groups:
        ps_bufs = min(ps_bufs, 6)   # leave 2 PSUM banks for PE transposes

    with tile.TileContext(nc) as tc:
        with (
            tc.tile_pool(name="wbt", bufs=1) as wbt_pool,
            tc.tile_pool(name="ps", bufs=ps_bufs, space="PSUM") as ps_pool,
        ):
            from contextlib import ExitStack
            _aux = ExitStack()
            ident = None
            if pe_groups:
                from concourse import masks
                ident_pool = _aux.enter_context(
                    tc.tile_pool(name="ident", bufs=1))
                ident = ident_pool.tile([P, P], BF16)
                masks.make_identity(nc, ident[:])
            # ---- Phase A: binarized, transposed weights (resident) ----
            # bf16 path: wbt[i_local, k, o] = (u < weight)[o, k*128 + i]
            # fp8 path:  the binarized row is cast to fp8e4 and the xbar
            #   transpose runs on a bf16 *view*, so each transposed 2-byte
            #   unit carries the (i=2q, i=2q+1) fp8 pair; the pair becomes
            #   DoubleRow's two k-planes via byte-strided APs. wbt8_bf
            #   [u, k', o] (bf16 units) = fp8 bytes wb[o, k'*256 + 2u + e].
            # One 2MB DMA per o-tile per tensor (chunking these into 32
            # small DMAs put ~80us of serialized SWDGE latency on the
            # critical path gating the first matmul); w rides SWDGE while
            # u rides the scalar HWDGE so descriptor generation overlaps.
            # wu/wb pools are phase-A-scoped; SBUF is reclaimed for B.
            if fp8:
                wbt8_bf = wbt_pool.tile([P, NKP, ONUM], BF16)
                wbt8_f8 = wbt8_bf[:].bitcast(F8)   # [P, NKP, 2*ONUM]
            else:
                wbt = wbt_pool.tile([P, NK, ONUM], BF16)
            with (
                tc.tile_pool(name="wu", bufs=2) as wu_pool,
                tc.tile_pool(name="wb", bufs=2) as wb_pool,
            ):
                for ot in range(NOT):
                    w_t = wu_pool.tile([P, INUM], F32, tag="w")
                    u_t = wu_pool.tile([P, INUM], F32, tag="u")
                    nc.gpsimd.dma_start(out=w_t[:],
                                        in_=w_d[ot * P:(ot + 1) * P, :])
                    nc.scalar.dma_start(out=u_t[:],
                                        in_=u_d[ot * P:(ot + 1) * P, :])
                    if fp8:
                        wb8_t = wb_pool.tile([P, INUM], F8, tag="wb8")
                        nc.vector.tensor_tensor(wb8_t[:], u_t[:], w_t[:],
                                                op=mybir.AluOpType.is_lt)
                        nc.sync.dma_start(
                            out=wbt8_bf[:, :, ot * P:(ot + 1) * P],
                            in_=wb8_t[:].bitcast(BF16), transpose=True)
                    else:
                        wb_t = wb_pool.tile([P, INUM], BF16, tag="wb")
                        nc.vector.tensor_tensor(wb_t[:], u_t[:], w_t[:],
                                                op=mybir.AluOpType.is_lt)
                        nc.sync.dma_start(
                            out=wbt[:, :, ot * P:(ot + 1) * P],
                            in_=wb_t[:], transpose=True)

            # ---- Phase B: stream batch tiles, gb tiles per DMA group ----
            with (
                tc.tile_pool(name="xn", bufs=xn_bufs) as xn_pool,
                tc.tile_pool(name="xt", bufs=xt_bufs) as xt_pool,
                tc.tile_pool(name="ob", bufs=ob_bufs) as ob_pool,
            ):
                pst_pool = None
                if pe_groups:
                    pst_pool = _aux.enter_context(
                        tc.tile_pool(name="pst", bufs=2, space="PSUM"))
                QT = 4   # k-tiles per PE-transpose PSUM staging quad

                def run_groups(_iv=None, prefetch=2):
                    # software-pipelined emission: group g+prefetch's
                    # load+transpose are emitted before group g's matmuls,
                    # so the prefetch DMAs outrank compute in the
                    # scheduler's priority order
                    xts = {}

                    def emit_fetch(g):
                        if fp8:
                            # xn8[p, j, i] = fp8(x[(g*gb + j)*P + p, i]),
                            # cast in the DMA; the xbar transpose then moves
                            # 2-byte units, i.e. (2q, 2q+1) fp8 pairs:
                            # xt8_bf[u, j*NKP + t, b] unit = fp8 bytes
                            # x[(g*gb+j)*P + b, t*256 + 2u + e]
                            xn8 = xn_pool.tile([P, gb, INUM], F8, tag="xn")
                            nc.gpsimd.dma_start(out=xn8[:], in_=x_v[g])
                            xt = xt_pool.tile([P, gb * NKP, P], BF16,
                                              tag="xt")
                            nc.sync.dma_start(out=xt[:],
                                              in_=xn8[:].bitcast(BF16),
                                              transpose=True)
                        else:
                            # xn[p, j, i] = x[(g*gb + j)*P + p, i]
                            xn = xn_pool.tile([P, gb, INUM], BF16, tag="xn")
                            nc.gpsimd.dma_start(out=xn[:], in_=x_v[g])
                            # xt[p, j*NK + k, f] = xn_2d[f, j*INUM + k*P + p]
                            #                    = x[(g*gb + j)*P + f, k*P+p]
                            xt = xt_pool.tile([P, gb * NK, P], BF16,
                                              tag="xt")
                            emit_transpose(g, xn, xt)
                        xts[g] = xt

                    ob = None
                    for g in range(min(prefetch + 1, ngrp)):
                        emit_fetch(g)
                    for g in range(ngrp):
                        xt = xts.pop(g)
                        for j in range(gb):
                            bt = g * gb + j
                            jj = bt % store_gb
                            if jj == 0:
                                ob = ob_pool.tile([P, store_gb, ONUM], F32,
                                                  tag="ob")
                            ps = ps_pool.tile([P, ONUM], F32, tag="ps")
                            if fp8:
                                # DoubleRowSwInterleave: the stationary
                                # operand is the RAW pair-interleaved 256-byte
                                # block the 2-byte xbar transpose produced
                                # (walrus rejects byte-strided Ldweights APs;
                                # SWI is the hw mode for interleaved weights).
                                # The hw reads weight columns last-first, so
                                # psum partition m holds batch row 127-m; the
                                # host un-reverses each 128-row block.
                                xt_f8 = xt[:].bitcast(F8)  # [P, gb*NKP, 256]
                                for t in range(NKP):
                                    lhsT = xt_f8[:, j * NKP + t, :].rearrange(
                                        "p (a b) -> p a b", a=2)
                                    rhs = wbt8_f8[:, t, :].rearrange(
                                        "p (o e) -> p e o", e=2)
                                    nc.tensor.matmul(
                                        ps[:], lhsT, rhs,
                                        start=(t == 0), stop=(t == NKP - 1),
                                        perf_mode=mybir.MatmulPerfMode
                                        .DoubleRowSwInterleave)
                            else:
                                for k in range(NK):
                                    nc.tensor.matmul(ps[:],
                                                     xt[:, j * NK + k, :],
                                                     wbt[:, k, :],
                                                     start=(k == 0),
                                                     stop=(k == NK - 1))
                            nc.vector.tensor_copy(ob[:, jj, :], ps[:])
                            if jj == store_gb - 1:
                                nc.scalar.dma_start(out=o_v[bt // store_gb],
                                                    in_=ob[:])
                        if g + prefetch + 1 < ngrp:
                            emit_fetch(g + prefetch + 1)

                def emit_transpose(g, xn, xt):
                    if g in pe_groups:
                        # tensor-engine transpose: each [128b,128i]
                        # sub-tile via identity matmul into bf16 PSUM,
                        # then DVE copies a quad back to SBUF
                        for j in range(gb):
                            for q in range(NK // QT):
                                pst = pst_pool.tile([P, QT, P], BF16,
                                                    tag="pst")
                                for h in range(QT):
                                    k = q * QT + h
                                    nc.tensor.transpose(
                                        pst[:, h, :],
                                        xn[:, j, k * P:(k + 1) * P],
                                        ident[:])
                                kk = j * NK + q * QT
                                nc.vector.tensor_copy(
                                    xt[:, kk:kk + QT, :], pst[:])
                    else:
                        nc.sync.dma_start(out=xt[:], in_=xn[:],
                                          transpose=True)

                if loop is None:
                    run_groups()
                else:
                    with tc.For_i(0, loop, 1):
                        run_groups()
            _aux.close()   # LIFO: release pst/ident after xn/xt/ob

    nc.compile()
    return nc


def _make_exec(nc):
    """Build a jitted shard_map executable over the 8 cores (mirrors
    bass2jax.run_bass_via_pjrt's multi-core path, without donation so the
    same device buffers can be re-executed for timing)."""
    import jax
    from jax.sharding import Mesh, PartitionSpec
    from jax.experimental.shard_map import shard_map
    from concourse import bass2jax

    bass2jax.install_neuronx_cc_hook()
    partition_name = (nc.partition_id_tensor.name
                      if nc.partition_id_tensor else None)
    in_names, out_names, out_avals = [], [], []
    for alloc in nc.m.functions[0].allocations:
        if not isinstance(alloc, mybir.MemoryLocationSet):
            continue
        name = alloc.memorylocations[0].name
        if alloc.kind == "ExternalInput":
            if name != partition_name:
                in_names.append(name)
        elif alloc.kind == "ExternalOutput":
            out_names.append(name)
            out_avals.append(jax.core.ShapedArray(
                tuple(alloc.tensor_shape), mybir.dt.np(alloc.dtype)))
    n_params = len(in_names)
    all_names = in_names + out_names
    if partition_name is not None:
        all_names = all_names + [partition_name]

    def _body(*args):
        operands = list(args)
        if partition_name is not None:
            operands.append(bass2jax.partition_id_tensor())
        return tuple(bass2jax._bass_exec_p.bind(
            *operands,
            out_avals=tuple(out_avals),
            in_names=tuple(all_names),
            out_names=tuple(out_names),
            lowering_input_output_aliases=(),
            sim_require_finite=True,
            sim_require_nnan=True,
            nc=nc,
        ))

    devices = jax.devices()[:NCORES]
    mesh = Mesh(np.asarray(devices), ("core",))

    def make_fn(reps):
        def _rep_body(*args):
            outs = None
            for _ in range(reps):
                outs = _body(*args)   # effectful primitive: not CSE'd
            return outs
        return jax.jit(
            shard_map(_rep_body, mesh=mesh,
                      in_specs=(PartitionSpec("core"),) * (n_params + len(out_names)),
                      out_specs=(PartitionSpec("core"),) * len(out_names),
                      check_rep=False),
            keep_unused=True,
        )

    return make_fn, mesh, in_names[:n_params], out_names, out_avals


def bench(x, weight, u, r_lo=32, r_hi=1024, iters=6, **build_kw):
    """Measure real device time for one kernel execution.

    The axon RPC jitter (tens of ms) swamps a single ~250us execution, and
    multiple identical bass_exec calls in one program get CSE'd. So we
    build two NEFF variants whose phase B repeats in an on-device For_i
    loop (r_lo and r_hi iterations) and difference the wall-clock minima:
    (t_hi - t_lo)/(r_hi - r_lo) is one full phase-B pass of device time.
    Phase A (binarize+transpose weights, ~25us, runs once) is added from
    its cost-model share."""
    import time
    import jax
    from jax.sharding import NamedSharding, PartitionSpec

    concat = {
        "x": np.ascontiguousarray(x, dtype=np.float32),
        "weight": np.concatenate([weight] * NCORES, axis=0),
        "u": np.concatenate([u] * NCORES, axis=0),
    }

    def run_variant(r):
        nc = build(loop=r, **build_kw)
        make_fn, mesh, in_names, out_names, out_avals = _make_exec(nc)
        sh = NamedSharding(mesh, PartitionSpec("core"))
        args = [jax.device_put(concat[n], sh) for n in in_names]
        zeros = [
            jax.device_put(
                np.zeros((NCORES * a.shape[0], *a.shape[1:]), a.dtype), sh)
            for a in out_avals
        ]
        fn = make_fn(1)
        jax.block_until_ready(fn(*args, *zeros))    # compile + warm
        best = float("inf")
        for _ in range(iters):
            t0 = time.perf_counter()
            jax.block_until_ready(fn(*args, *zeros))
            best = min(best, time.perf_counter() - t0)
        return best

    t_lo = run_variant(r_lo)
    t_hi = run_variant(r_hi)
    pass_ns = (t_hi - t_lo) / (r_hi - r_lo) * 1e9
    phase_a_ns = 25_000.0   # one-time weight binarize+transpose (cost model)
    print(f"bench: loop{r_lo}={t_lo*1e3:.1f}ms loop{r_hi}={t_hi*1e3:.1f}ms "
          f"-> phase-B pass {pass_ns/1e3:.1f}us + phase-A ~{phase_a_ns/1e3:.0f}us")
    return pass_ns + phase_a_ns


def kernel(x, weight, u):
    x = np.ascontiguousarray(np.asarray(x), dtype=np.float32)
    weight = np.ascontiguousarray(np.asarray(weight), dtype=np.float32)
    u = np.ascontiguousarray(np.asarray(u), dtype=np.float32)
    assert x.shape == (B, INUM) and weight.shape == (ONUM, INUM)

    nc = _CACHE.get("nc")
    if nc is None:
        nc = _CACHE["nc"] = build()

    in_maps = [
        {"x": x[c * BLOC:(c + 1) * BLOC], "weight": weight, "u": u}
        for c in range(NCORES)
    ]
    res = run_bass_kernel_spmd(nc, in_maps, list(range(NCORES)))
    out = np.concatenate([res.results[c]["out"] for c in range(NCORES)],
                         axis=0)
    # SWI matmuls emit each 128-row batch tile with rows reversed
    out = np.ascontiguousarray(
        out.reshape(-1, P, ONUM)[:, ::-1, :].reshape(B, ONUM))
    return out

